# revision 13
# baseline (speedup 1.0000x reference)
"""BatchHardTriplet loss kernel for Trainium2 (8 NeuronCores, SPMD).

Strategy (v2)
-------------
Rows are sorted by label on the host; each core owns 1024 rows (8 chunks of
128) and computes its [128, 8192] sim block per chunk against an all-gathered,
per-core column-rotated operand so all positives land in narrow per-chunk
windows [wb+112, wb+400), wb = 128*mc.  The eq-mask (-2 where labels match,
incl. diagonal) is accumulated into the window region on the TensorEngine via
an identity @ mask_fp8 matmul, so hardest-negative retirement can scan whole
tiles.

v2 changes vs v1:
  - hardest-positive is computed EXACTLY on the host (classes are ~16 rows;
    per-class gram matrices cost ~10ms numpy) -> no window-min DVE work.
  - all input DMA issues live on the Sync queue; Scalar only runs
    activations (v1 burned ~6.5us of ScalarE on DMA_DIRECT2D issue).
  - column blocks are processed in order [2..7, 0, 1] so the masked blocks
    (cb 0/1) come last: mask/iden DMAs are never on the critical path and
    the first matmul only waits for ~300KB of input.
  - exp table-load (~2.7us) is triggered by a warmup activation during the
    DMA ramp.
  - retirement rebalanced by measured cost (DVE reduce 1002ns/tile vs ACT
    1484ns incl accumulator read): 38 tiles on DVE, 26 on ACT-LSE.
  - masks trimmed to the asserted 288-col window; 4 matmuls share each
    LDWEIGHTS (cb pairs per chunk).
"""

import sys
import numpy as np

sys.path.insert(0, "/opt/trn_rl_repo")

B = 8192
D = 128
M = 8            # cores
R = B // M       # 1024 rows per core
MC = R // 128    # 8 chunks of 128 rows per core
MARGIN = 0.3
MASKV = -2.0     # mask value added to label-equal sims
TAU = 80.0       # LSE sharpness (exp(80*sim) <= e^80 < bf16 max)
DELTA = 192      # rotation offset: chunk windows at [128*mc+112, 128*mc+400)
WLO, WHI = 112, 400
WN = WHI - WLO   # 288 mask cols per chunk
NDVE = 8         # dve max slots per chunk
NACT = 5         # lse slots per chunk
NS = NDVE + NACT
N_ACT_UNITS = 30  # of 64 [128,1024] tiles retired via ScalarE LSE

_CACHE = {}


def _unit_order():
    """Production order of the 64 (cb, mc) tiles: cb outer in order
    [2..7, 0, 1] (masked blocks last), mc inner — so each column block's
    DMA is only needed ~5us after the previous one."""
    order = []
    for cb in (2, 3, 4, 5, 6, 7, 0, 1):
        for mc in range(MC):
            order.append((cb, mc))
    return order


def _act_flags():
    """Strictly alternating ACT/DVE assignment (parity of cb-round + mc)
    so the two retirement engines interleave tile-by-tile; the first and
    last units are forced to DVE (DMA ramp / queue drain), with one
    mid-stream promotion to keep the 30/34 split."""
    flags = [((8 * ci + mc) and ((ci + mc) % 2 == 0))
             for ci in range(8) for mc in range(MC)]
    flags[0] = flags[2] = flags[63] = False
    flags[33] = True
    assert sum(flags) == N_ACT_UNITS
    return flags


def _build_program():
    if "nc" in _CACHE:
        return _CACHE["nc"]

    import concourse.bass as bass
    import concourse.bacc as bacc
    import concourse.mybir as mybir
    from concourse import tile

    f32 = mybir.dt.float32
    bf16 = mybir.dt.bfloat16
    fp8 = mybir.dt.float8e4
    Exp = mybir.ActivationFunctionType.Exp
    MAX = mybir.AluOpType.max
    X = mybir.AxisListType.X

    nc = bacc.Bacc(None, target_bir_lowering=False)

    embA = nc.dram_tensor("embA", [D, B], bf16, kind="ExternalInput")
    embB = nc.dram_tensor("embB", [D, R], bf16, kind="ExternalInput")
    masks = nc.dram_tensor("masks", [128, MC, WN], fp8, kind="ExternalInput")
    iden = nc.dram_tensor("iden", [128, 128], fp8, kind="ExternalInput")
    outs = nc.dram_tensor("outs", [128, MC, NS], f32, kind="ExternalOutput")

    flags = _act_flags()
    order = _unit_order()

    with tile.TileContext(nc) as tc:
        with (
            tc.tile_pool(name="big", bufs=1) as big,
            tc.tile_pool(name="ps", bufs=4, space="PSUM") as ps,
            tc.tile_pool(name="jk", bufs=4) as jk,
            tc.tile_pool(name="st", bufs=1) as st,
        ):
            # ---- warmup: trigger the exp table load during the DMA ramp
            warm = st.tile([128, 2], f32)
            nc.vector.memset(warm[:], 0.0)
            nc.scalar.activation(warm[:, 1:2], warm[:, 0:1], Exp, scale=1.0)

            # ---- input DMA: all issues on the Sync queue, first-needed first
            # Sync carries the ramp-critical stream; Vector issues two
            # mid-stream blocks before its first reduce; Scalar stays
            # clean so the exp-table load finishes early.
            Bt = big.tile([D, R], bf16)
            nc.sync.dma_start(Bt[:, 0:128], embB[:, 0:128])
            A = big.tile([D, B], bf16)
            nc.sync.dma_start(A[:, 2048:2560], embA[:, 2048:2560])
            nc.sync.dma_start(A[:, 2560:3072], embA[:, 2560:3072])
            nc.sync.dma_start(Bt[:, 128:R], embB[:, 128:R])
            nc.sync.dma_start(A[:, 3072:4096], embA[:, 3072:4096])
            nc.sync.dma_start(A[:, 4096:5120], embA[:, 4096:5120])
            out_t = st.tile([128, MC, NS], f32)
            nc.vector.memset(out_t[:], 0.0)
            nc.sync.dma_start(A[:, 5120:6144], embA[:, 5120:6144])
            nc.sync.dma_start(A[:, 6144:7168], embA[:, 6144:7168])
            nc.sync.dma_start(A[:, 7168:8192], embA[:, 7168:8192])
            Id = big.tile([128, 128], fp8)
            nc.sync.dma_start(Id[:], iden[:])
            Mk = big.tile([128, MC, WN], fp8)
            nc.sync.dma_start(Mk[:], masks[:])
            nc.sync.dma_start(A[:, 0:1024], embA[:, 0:1024])
            nc.sync.dma_start(A[:, 1024:2048], embA[:, 1024:2048])

            n_dve = [0] * MC
            n_act = [0] * MC
            for i, (cb, mc) in enumerate(order):
                lhsT = Bt[:, mc * 128:(mc + 1) * 128]
                P = ps.tile([128, 1024], f32, tag="psum", name=f"P{cb}_{mc}")
                for t in range(2):
                    lo = cb * 1024 + t * 512
                    nc.tensor.matmul(
                        P[:, t * 512:(t + 1) * 512],
                        lhsT,
                        A[:, lo:lo + 512],
                        start=True,
                        stop=True,
                    )
                # eq-mask (-2) into the window cols of this chunk, split at
                # PSUM bank (512) boundaries.  Window = [wb+112, wb+400)
                # global; cb0 holds [wb+112, min(1024, wb+400)), cb1 the
                # spill [1024, wb+400) for mc >= 5.
                wb = 128 * mc
                pieces = []
                if cb == 0:
                    g0, g1 = wb + WLO, min(wb + WHI, 1024)
                    while g0 < g1:
                        ge = min((g0 // 512 + 1) * 512, g1)
                        pieces.append((g0, ge, g0 - (wb + WLO)))
                        g0 = ge
                elif cb == 1 and wb + WHI > 1024:
                    # local coords within cb1 tile
                    ln = wb + WHI - 1024
                    pieces.append((1024, 1024 + ln, 1024 - (wb + WLO)))
                for (g0, g1, moff) in pieces:
                    nc.tensor.matmul(
                        P[:, g0 - cb * 1024:g1 - cb * 1024],
                        Id[:],
                        Mk[:, mc, moff:moff + (g1 - g0)],
                        start=False,
                        stop=True,
                        skip_group_check=True,
                    )
                if flags[i]:
                    j = jk.tile([128, 1024], bf16, tag="jk", name=f"j{i}")
                    nc.scalar.activation(
                        j[:], P[:], Exp, scale=TAU,
                        accum_out=out_t[:, mc, NDVE + n_act[mc]:
                                        NDVE + n_act[mc] + 1])
                    n_act[mc] += 1
                else:
                    nc.vector.tensor_reduce(
                        out_t[:, mc, n_dve[mc]:n_dve[mc] + 1], P[:],
                        axis=X, op=MAX)
                    n_dve[mc] += 1

            nc.sync.dma_start(outs[:], out_t[:])

    nc.compile()
    _CACHE["nc"] = nc
    return nc


def _prep_inputs(emb, labels):
    """Sort by label, build per-core rotated operands + fp8 masks, and
    compute hardest-positive distances exactly on the host."""
    import ml_dtypes

    emb = np.asarray(emb, dtype=np.float32)
    labels = np.asarray(labels)
    order = np.argsort(labels, kind="stable")
    labs = labels[order]
    embs = emb[order]
    embT = np.ascontiguousarray(embs.T)  # [D, B]

    starts = np.searchsorted(labs, labs, side="left")
    ends = np.searchsorted(labs, labs, side="right")
    counts = ends - starts
    valid = (counts >= 2) & (counts < B)

    # hardest positive (max distance over same-label pairs), exact fp32
    hp = np.zeros(B, dtype=np.float32)
    run_starts = np.unique(starts)
    for s in run_starts:
        e = int(ends[s])
        n = e - s
        if n < 2:
            continue
        Es = embs[s:e]
        G = Es @ Es.T
        np.fill_diagonal(G, np.inf)
        hp[s:e] = 1.0 - G.min(axis=1)

    iden = np.eye(128, dtype=ml_dtypes.float8_e4m3)

    in_maps = []
    for c in range(M):
        r0 = c * R
        s = int(starts[r0])
        for mc in range(MC):
            rr0 = r0 + mc * 128
            lo = int(starts[rr0]) - s + DELTA
            hi = int(ends[rr0 + 127]) - s + DELTA
            assert 128 * mc + WLO <= lo and hi <= 128 * mc + WHI, (
                f"chunk window [{lo},{hi}) outside "
                f"[{128*mc+WLO},{128*mc+WHI})"
            )
        perm = (s - DELTA + np.arange(B)) % B
        embA = np.ascontiguousarray(embT[:, perm]).astype(ml_dtypes.bfloat16)
        embB = np.ascontiguousarray(
            embT[:, r0:r0 + R]).astype(ml_dtypes.bfloat16)
        lab_rows = labs[r0:r0 + R].reshape(MC, 128)
        win_cols = (128 * np.arange(MC)[:, None] + WLO
                    + np.arange(WN)[None, :])
        lab_win = labs[perm[win_cols]]                        # [MC, WN]
        eq = lab_rows[:, :, None] == lab_win[:, None, :]      # [MC, 128, WN]
        mk = np.where(eq, np.float32(MASKV), np.float32(0.0)).astype(
            ml_dtypes.float8_e4m3
        )
        mk = np.ascontiguousarray(mk.transpose(1, 0, 2))
        in_maps.append(
            {"embA": embA, "embB": embB, "masks": mk, "iden": iden}
        )
    return in_maps, (valid, hp)


def _postprocess(results, aux):
    valid, hp = aux
    flags = _act_flags()
    order = _unit_order()
    n_dve = np.zeros(MC, dtype=int)
    n_act = np.zeros(MC, dtype=int)
    for i, (cb, mc) in enumerate(order):
        if flags[i]:
            n_act[mc] += 1
        else:
            n_dve[mc] += 1

    hn_sim = np.zeros(B, dtype=np.float64)
    for c, res in enumerate(results):
        o = res["outs"].astype(np.float64)  # [128, MC, NS]
        for mc in range(MC):
            mx = -np.inf
            if n_dve[mc] > 0:
                mx = o[:, mc, :n_dve[mc]].max(axis=1)
            if n_act[mc] > 0:
                se = o[:, mc, NDVE:NDVE + n_act[mc]].sum(axis=1)
                lse = np.log(np.maximum(se, 1e-300)) / TAU
                mx = np.maximum(mx, lse)
            rows = slice(c * R + mc * 128, c * R + mc * 128 + 128)
            hn_sim[rows] = mx
    hn = 1.0 - hn_sim
    per_row = np.maximum(0.0, hp - hn + MARGIN)
    cnt = int(valid.sum())
    if cnt == 0:
        return np.float32(0.0)
    return np.float32(np.sum(per_row[valid]) / cnt)


def run_device(in_maps, trace=False):
    from concourse.bass_utils import run_bass_kernel_spmd

    nc = _build_program()
    return run_bass_kernel_spmd(nc, in_maps, list(range(M)), trace=trace)


def kernel(emb, labels):
    in_maps, aux = _prep_inputs(emb, labels)
    out = run_device(in_maps, trace=False)
    return _postprocess(out.results, aux)


if __name__ == "__main__":
    rng = np.random.default_rng(0)
    emb = rng.standard_normal((B, D)).astype(np.float32)
    emb /= np.linalg.norm(emb, axis=1, keepdims=True) + 1e-12
    labels = rng.integers(0, 512, B).astype(np.int32)
    print(kernel(emb, labels))
